# revision 42
# baseline (speedup 1.0000x reference)
"""CrossAttentionBlock3D on 8 TRN2 NeuronCores — sequence-parallel Bass kernel.

Sharding: the 32768 spatial tokens are split 8x4096 across cores. GroupNorm
statistics are the only cross-core dependency (one 64-float AllReduce).
Everything else (LN, K/V projections over the tiny context, Q/attention/proj
for the local tokens) is computed locally; context-side work is replicated.

Compute dtype: bf16 matmuls with f32 PSUM accumulation (validated end-to-end
rel err ~3e-3 vs the f32 reference).
"""
import sys

sys.path.insert(0, "/opt/trn_rl_repo")

import numpy as np
import ml_dtypes

from concourse import bass, bacc, tile, mybir, masks
from concourse.bass_utils import run_bass_kernel_spmd

F32 = mybir.dt.float32
BF16 = mybir.dt.bfloat16
BF = ml_dtypes.bfloat16

B, C, D, H, W = 2, 512, 32, 32, 32
S = D * H * W              # 32768
L, CTX = 256, 768
NH, HD, G = 8, 64, 8
EPS = 1e-5
NCORES = 8
CT, OT = 4, 4              # channel tiles (C = 4*128)
KCT = 6                    # ctx channel tiles (CTX = 6*128)
LT = 2                     # L = 2*128


def build(nc, s_loc, st_size):
    """Emit the per-core Tile program. s_loc: local tokens; st_size: S-tile."""
    nst = s_loc // st_size
    n_gn = (C // G) * S    # global elems per (b, g) group

    WCOLS = 2 * CT * C + 2 * KCT * C          # qw | kw | vw | pw packed
    BCOLS = 3 * CT + 2 * B * CT               # qb | kb | pb | gnw8 | gnb8
    x_in = nc.dram_tensor("x", [B, CT, 128, s_loc], BF16, kind="ExternalInput")
    ctx_in = nc.dram_tensor("ctx", [128, B * LT * CTX], F32, kind="ExternalInput")
    w_in = nc.dram_tensor("wpack", [128, WCOLS], BF16, kind="ExternalInput")
    b_in = nc.dram_tensor("bpack", [128, BCOLS], F32, kind="ExternalInput")
    sel_in = nc.dram_tensor("sel8", [8, 8 * 64], BF16, kind="ExternalInput")
    rm_in = nc.dram_tensor("rmask", [64, 8], F32, kind="ExternalInput")
    out_ext = nc.dram_tensor("out", [B, nst, 128, CT * st_size], F32,
                             kind="ExternalOutput")

    from contextlib import ExitStack
    with tile.TileContext(nc) as tc, ExitStack() as es:
        wp = es.enter_context(tc.tile_pool(name="wp", bufs=1))
        dram = es.enter_context(tc.tile_pool(name="dram", bufs=1, space="DRAM"))

        # ---- persistent SBUF tensors ----
        w_all = wp.tile([128, WCOLS], BF16, tag="wpack")
        b_all = wp.tile([128, BCOLS], F32, tag="bpack")
        qw_t = w_all[:, 0:CT * C]
        kw_t = w_all[:, CT * C:CT * C + KCT * C]
        vw_t = w_all[:, CT * C + KCT * C:CT * C + 2 * KCT * C]
        pw_t = w_all[:, CT * C + 2 * KCT * C:WCOLS]
        qb_t = b_all[:, 0:CT]
        kb_t = b_all[:, CT:2 * CT]
        pb_t = b_all[:, 2 * CT:3 * CT]
        gnw_t = b_all[:, 3 * CT:3 * CT + B * CT]
        gnb_t = b_all[:, 3 * CT + B * CT:BCOLS]
        ctxT_all = wp.tile([128, B * KCT * L], BF16, tag="ctxT")
        kT_all = wp.tile([128, B * CT * L], BF16, tag="kT")
        v_all = wp.tile([128, B * LT * (NH * (HD + 1))], BF16, tag="v")
        ones_t = wp.tile([1, 64], F32, tag="ones")
        ident = wp.tile([128, 128], BF16, tag="ident")
        stats_s = wp.tile([128, 16], F32, tag="stats")
        h_all = wp.tile([128, B * CT * s_loc], BF16, tag="h_all")
        a_pc = wp.tile([128, B * CT], F32, tag="a_pc")
        bias_pc = wp.tile([128, B * CT], F32, tag="bias_pc")

        sel8_t = wp.tile([8, 8 * 64], BF16, tag="sel8")
        rmask_t = wp.tile([64, 8], F32, tag="rmask")
        nc.vector.memset(ones_t[:], 1.0)
        masks.make_identity(nc, ident[:])

        # ---- GroupNorm partial stats (local) ----
        with tc.tile_pool(name="setup", bufs=1) as sp, \
             tc.tile_pool(name="setup_ps", bufs=2, space="PSUM") as spp:
            warm_in = dram.tile([8, 4], F32, tag="warm_in")
            warm_out = dram.tile([64, 4], F32, tag="warm_out")
            warm_s = sp.tile([8, 4], F32, tag="warm_s")
            nc.vector.memset(warm_s[:], 0.0)
            nc.gpsimd.dma_start(warm_in[:], warm_s[:])
            nc.gpsimd.collective_compute(
                "AllGather", mybir.AluOpType.bypass,
                replica_groups=[list(range(NCORES))],
                ins=[warm_in.opt()], outs=[warm_out.opt()])
            for b in range(B):
                for t in range(CT):
                    col = b * CT + t
                    x_t = sp.tile([128, s_loc], BF16, tag="x_t", bufs=3,
                                  name=f"x_t_{col}")
                    nc.sync.dma_start(x_t[:], x_in[b, t])
                    nc.vector.tensor_reduce(
                        stats_s[:, col:col + 1], x_t[:], mybir.AxisListType.X,
                        mybir.AluOpType.add)
                    nc.scalar.activation(
                        h_all[:, col * s_loc:(col + 1) * s_loc], x_t[:],
                        mybir.ActivationFunctionType.Square,
                        accum_out=stats_s[:, 8 + col:9 + col])
            nc.sync.dma_start(w_all[:], w_in[:])
            nc.sync.dma_start(b_all[:], b_in[:])
            nc.sync.dma_start(sel8_t[:], sel_in[:])
            nc.sync.dma_start(rmask_t[:], rm_in[:])
            ctxf = sp.tile([128, B * LT * CTX], F32, tag="ctxf")
            nc.sync.dma_start(ctxf[:], ctx_in[:])

            mask2 = sp.tile([128, 2], F32, tag="mask2")
            nc.vector.memset(mask2[:, :], 0.0)
            nc.vector.memset(mask2[0:64, 0:1], 1.0)
            nc.vector.memset(mask2[64:128, 1:2], 1.0)
            st_p = spp.tile([8, 4], F32, tag="st_p", bufs=1)
            nc.tensor.matmul(st_p[:, 0:2], stats_s[:, 0:8], mask2[:],
                             start=True, stop=True)
            nc.tensor.matmul(st_p[:, 2:4], stats_s[:, 8:16], mask2[:],
                             start=True, stop=True)
            red_s = sp.tile([8, 4], F32, tag="red_s")
            nc.vector.tensor_copy(red_s[:], st_p[:])

            cc_in = dram.tile([8, 4], F32, tag="cc_in")
            cc_ag = dram.tile([64, 4], F32, tag="cc_ag")
            nc.gpsimd.dma_start(cc_in[:], red_s[:])
            nc.gpsimd.collective_compute(
                "AllGather", mybir.AluOpType.bypass,
                replica_groups=[list(range(NCORES))],
                ins=[cc_in.opt()], outs=[cc_ag.opt()])
            ag_s = sp.tile([64, 4], F32, tag="ag_s")
            nc.gpsimd.dma_start(ag_s[:], cc_ag[:])
            sum_p = spp.tile([2, 8], F32, tag="sum_p", bufs=1)
            sq_p = spp.tile([2, 8], F32, tag="sq_p", bufs=1)
            nc.tensor.matmul(sum_p[:], ag_s[:, 0:2], rmask_t[:],
                             start=True, stop=True)
            nc.tensor.matmul(sq_p[:], ag_s[:, 2:4], rmask_t[:],
                             start=True, stop=True)

            # per-(b,g) mean / rstd, laid out [2 halves, 8 (b,t)]
            mu8 = sp.tile([2, 8], F32, tag="mu8")
            rstd8 = sp.tile([2, 8], F32, tag="rstd8")
            ex28 = sp.tile([2, 8], F32, tag="ex28")
            var8 = sp.tile([2, 8], F32, tag="var8")
            sd8 = sp.tile([2, 8], F32, tag="sd8")
            eps8 = sp.tile([2, 1], F32, tag="eps8")
            nc.vector.memset(eps8[:], EPS)
            nc.vector.tensor_scalar_mul(mu8[:], sum_p[:], 1.0 / n_gn)
            nc.vector.tensor_scalar_mul(ex28[:], sq_p[:], 1.0 / n_gn)
            nc.vector.scalar_tensor_tensor(
                var8[:], mu8[:], -1.0, mu8[:],
                mybir.AluOpType.mult, mybir.AluOpType.mult)
            nc.vector.tensor_add(var8[:], var8[:], ex28[:])
            nc.scalar.activation(sd8[:], var8[:],
                                 mybir.ActivationFunctionType.Sqrt, bias=eps8[:])
            nc.vector.reciprocal(rstd8[:], sd8[:])

            # broadcast [2,8] -> [128, 8] via DRAM bounce + stride-0 DMA
            mu_d = dram.tile([2, 8], F32, tag="mu_d")
            rstd_d = dram.tile([2, 8], F32, tag="rstd_d")
            nc.gpsimd.dma_start(mu_d[:], mu8[:])
            nc.gpsimd.dma_start(rstd_d[:], rstd8[:])
            mu_bc = sp.tile([128, 8], F32, tag="mu_bc")
            rstd_bc = sp.tile([128, 8], F32, tag="rstd_bc")
            nc.gpsimd.dma_start(
                mu_bc[:], mu_d[:, :].unsqueeze(1).broadcast_to((2, 64, 8)))
            nc.gpsimd.dma_start(
                rstd_bc[:], rstd_d[:, :].unsqueeze(1).broadcast_to((2, 64, 8)))

            # per-channel affine: h = a*x + bias
            nc.vector.tensor_mul(a_pc[:], rstd_bc[:], gnw_t[:])
            tmp_bc = sp.tile([128, 8], F32, tag="tmp_bc")
            nc.vector.tensor_mul(tmp_bc[:], mu_bc[:], a_pc[:])
            nc.vector.tensor_sub(bias_pc[:], gnb_t[:], tmp_bc[:])

            # ---- h = a*x + bias for the whole shard (bf16) ----
            # b0 first-tile slices first so q(0,0) unblocks immediately
            for b in range(B):
                for ct in range(CT):
                    col = b * CT + ct
                    x_t2 = sp.tile([128, s_loc], BF16, tag="x_t", bufs=3,
                                   name=f"x_t2_{col}")
                    nc.sync.dma_start(x_t2[:], x_in[b, ct])
                    if b == 0:
                        nc.vector.tensor_scalar(
                            h_all[:, col * s_loc:col * s_loc + st_size],
                            x_t2[:, 0:st_size],
                            a_pc[:, col:col + 1], bias_pc[:, col:col + 1],
                            mybir.AluOpType.mult, mybir.AluOpType.add)
                        nc.vector.tensor_scalar(
                            h_all[:, col * s_loc + st_size:(col + 1) * s_loc],
                            x_t2[:, st_size:],
                            a_pc[:, col:col + 1], bias_pc[:, col:col + 1],
                            mybir.AluOpType.mult, mybir.AluOpType.add)
                    else:
                        nc.vector.tensor_scalar(
                            h_all[:, col * s_loc:(col + 1) * s_loc],
                            x_t2[:],
                            a_pc[:, col:col + 1], bias_pc[:, col:col + 1],
                            mybir.AluOpType.mult, mybir.AluOpType.add)

            # ---- context path: LN + transpose + K/V ----
            ctxn = sp.tile([128, B * LT * CTX], BF16, tag="ctxn")
            eps128 = sp.tile([128, 1], F32, tag="eps128")
            nc.vector.memset(eps128[:], EPS)
            for b in range(B):
                for lt in range(LT):
                    cs = ctxf[:, (b * LT + lt) * CTX:(b * LT + lt + 1) * CTX]
                    cs1 = sp.tile([128, 1], F32, tag="cs1", bufs=2)
                    cs2 = sp.tile([128, 1], F32, tag="cs2", bufs=2)
                    csq = sp.tile([128, CTX], F32, tag="csq", bufs=2)
                    nc.vector.tensor_reduce(cs1[:], cs, mybir.AxisListType.X,
                                            mybir.AluOpType.add)
                    nc.scalar.activation(csq[:], cs,
                                         mybir.ActivationFunctionType.Square,
                                         accum_out=cs2[:])
                    cmu = sp.tile([128, 1], F32, tag="cmu", bufs=2)
                    cex2 = sp.tile([128, 1], F32, tag="cex2", bufs=2)
                    cvar = sp.tile([128, 1], F32, tag="cvar", bufs=2)
                    csd = sp.tile([128, 1], F32, tag="csd", bufs=2)
                    crstd = sp.tile([128, 1], F32, tag="crstd", bufs=2)
                    cnm = sp.tile([128, 1], F32, tag="cnm", bufs=2)
                    nc.vector.tensor_scalar_mul(cmu[:], cs1[:], 1.0 / CTX)
                    nc.vector.tensor_scalar_mul(cex2[:], cs2[:], 1.0 / CTX)
                    nc.vector.scalar_tensor_tensor(
                        cvar[:], cmu[:], -1.0, cmu[:],
                        mybir.AluOpType.mult, mybir.AluOpType.mult)
                    nc.vector.tensor_add(cvar[:], cvar[:], cex2[:])
                    nc.scalar.activation(csd[:], cvar[:],
                                         mybir.ActivationFunctionType.Sqrt,
                                         bias=eps128[:])
                    nc.vector.reciprocal(crstd[:], csd[:])
                    nc.vector.scalar_tensor_tensor(
                        cnm[:], cmu[:], -1.0, crstd[:],
                        mybir.AluOpType.mult, mybir.AluOpType.mult)
                    nc.vector.tensor_scalar(
                        ctxn[:, (b * LT + lt) * CTX:(b * LT + lt + 1) * CTX],
                        cs, crstd[:], cnm[:],
                        mybir.AluOpType.mult, mybir.AluOpType.add)

            # transpose ctxn -> ctxT_all  [128ctx, L] per (b, kct)
            for b in range(B):
                for lt in range(LT):
                    for ct in range(KCT):
                        tp_p = spp.tile([128, 128], BF16, tag="tp_p")
                        nc.tensor.transpose(
                            tp_p[:],
                            ctxn[:, (b * LT + lt) * CTX + ct * 128:
                                 (b * LT + lt) * CTX + (ct + 1) * 128],
                            ident[:])
                        nc.scalar.copy(
                            ctxT_all[:, (b * KCT + ct) * L + lt * 128:
                                     (b * KCT + ct) * L + (lt + 1) * 128],
                            tp_p[:])

            # kT[b, ot] [128, L]
            for b in range(B):
                for ot in range(OT):
                    k_p = spp.tile([128, L], F32, tag="k_p", bufs=1)
                    for ct in range(KCT):
                        nc.tensor.matmul(
                            k_p[:],
                            kw_t[:, ct * C + ot * 128:ct * C + (ot + 1) * 128],
                            ctxT_all[:, (b * KCT + ct) * L:(b * KCT + ct + 1) * L],
                            start=(ct == 0), stop=(ct == KCT - 1))
                    nc.scalar.activation(
                        kT_all[:, (b * CT + ot) * L:(b * CT + ot + 1) * L],
                        k_p[:], mybir.ActivationFunctionType.Identity,
                        bias=kb_t[:, ot:ot + 1])

            # v'[b, lt] [128, NH*(HD+1)]  (per-head ones column appended)
            VW = NH * (HD + 1)
            for b in range(B):
                for lt in range(LT):
                    v_p = spp.tile([128, C], F32, tag="v_p", bufs=1)
                    for ct in range(KCT):
                        nc.tensor.matmul(
                            v_p[:],
                            ctxT_all[:, (b * KCT + ct) * L + lt * 128:
                                     (b * KCT + ct) * L + (lt + 1) * 128],
                            vw_t[:, ct * C:(ct + 1) * C],
                            start=(ct == 0), stop=(ct == KCT - 1))
                    vs = v_all[:, (b * LT + lt) * VW:(b * LT + lt + 1) * VW]
                    nc.scalar.copy(
                        vs.rearrange("p (h e) -> p h e", e=HD + 1)[:, :, 0:HD],
                        v_p[:])
                    nc.vector.memset(
                        vs.rearrange("p (h e) -> p h e", e=HD + 1)[:, :, HD:HD + 1],
                        1.0)

        # ---- main attention loop (software-pipelined) ----
        with tc.tile_pool(name="mp", bufs=2) as mp, \
             tc.tile_pool(name="op", bufs=3) as op, \
             tc.tile_pool(name="mm_ps", bufs=2, space="PSUM") as mmp, \
             tc.tile_pool(name="z_ps", bufs=2, space="PSUM") as zp, \
             tc.tile_pool(name="o_ps", bufs=1, space="PSUM") as opp, \
             tc.tile_pool(name="rsb_ps", bufs=1, space="PSUM") as rbp, \
             tc.tile_pool(name="rs_dram", bufs=3, space="DRAM") as rsd:

            def emit_q(b, st):
                lo = st * st_size
                q_s = mp.tile([128, CT * st_size], BF16, tag="q_s", bufs=3,
                              name=f"q_s_{b}_{st}")
                for ot in range(OT):
                    q_p = mmp.tile([128, st_size], F32, tag="mm_p",
                                   name=f"q_p_{b}_{st}_{ot}")
                    for ct in range(CT):
                        nc.tensor.matmul(
                            q_p[:],
                            qw_t[:, ct * C + ot * 128:ct * C + (ot + 1) * 128],
                            h_all[:, (b * CT + ct) * s_loc + lo:
                                  (b * CT + ct) * s_loc + lo + st_size],
                            start=(ct == 0), stop=(ct == CT - 1))
                    nc.scalar.activation(
                        q_s[:, ot * st_size:(ot + 1) * st_size], q_p[:],
                        mybir.ActivationFunctionType.Identity,
                        bias=qb_t[:, ot:ot + 1])
                return q_s

            def emit_head_group(b, st, hg, q_s, rs8_d, o_list):
                for hj in range(4):
                    hh = hg * 4 + hj
                    ko, po = hh // 2, (hh % 2) * 64
                    p_t = mp.tile([128, 2 * st_size], BF16, tag="p_t", bufs=3,
                                  name=f"p_t_{b}_{st}_{hh}")
                    z_p = zp.tile([128, 2 * st_size], F32, tag="z_p",
                                  name=f"z_p_{b}_{st}_{hh}")
                    for lh in range(LT):
                        nc.tensor.matmul(
                            z_p[:, lh * st_size:(lh + 1) * st_size],
                            kT_all[po:po + 64,
                                   (b * CT + ko) * L + lh * 128:
                                   (b * CT + ko) * L + (lh + 1) * 128],
                            q_s[po:po + 64, ko * st_size:(ko + 1) * st_size],
                            start=True, stop=True)
                    nc.scalar.activation(p_t[:], z_p[:],
                                         mybir.ActivationFunctionType.Exp)
                    o_p = opp.tile([HD + 1, st_size], F32, tag="o_p",
                                   name=f"o_p_{b}_{st}_{hh}")
                    for lh in range(LT):
                        nc.tensor.matmul(
                            o_p[:],
                            v_all[:, (b * LT + lh) * VW + hh * (HD + 1):
                                  (b * LT + lh) * VW + (hh + 1) * (HD + 1)],
                            p_t[:, lh * st_size:(lh + 1) * st_size],
                            start=(lh == 0), stop=(lh == LT - 1))
                    o_s = mp.tile([HD + 1, st_size], F32, tag="o_s", bufs=10,
                                  name=f"o_s_{b}_{st}_{hh}")
                    if hj % 2 == 0:
                        nc.scalar.copy(o_s[:], o_p[:])
                    else:
                        nc.vector.tensor_copy(o_s[:], o_p[:])
                    o_list.append(o_s)
                    nc.gpsimd.dma_start(rs8_d[hh:hh + 1, :], o_s[HD:HD + 1, :])

            def emit_normalize(b, st, rs8_d, o_list, proj_rhs):
                rs8_s = mp.tile([8, st_size], F32, tag="rs8_s",
                                name=f"rs8_s_{b}_{st}")
                nc.gpsimd.dma_start(rs8_s[:], rs8_d[:])
                rec8 = mp.tile([8, st_size], F32, tag="rec8",
                               name=f"rec8_{b}_{st}")
                nc.vector.reciprocal(rec8[:], rs8_s[:])
                rec8b = mp.tile([8, st_size], BF16, tag="rec8b",
                                name=f"rec8b_{b}_{st}")
                nc.vector.tensor_copy(rec8b[:], rec8[:])
                for hh in range(NH):
                    ko, po = hh // 2, (hh % 2) * 64
                    rsb_p = rbp.tile([64, st_size], F32, tag="rsb_p",
                                     name=f"rsb_p_{b}_{st}_{hh}")
                    nc.tensor.matmul(
                        rsb_p[:], sel8_t[:, hh * 64:(hh + 1) * 64],
                        rec8b[:], start=True, stop=True)
                    nc.vector.tensor_tensor(
                        proj_rhs[po:po + 64, ko * st_size:(ko + 1) * st_size],
                        o_list[hh][0:HD, :], rsb_p[:], mybir.AluOpType.mult)

            def emit_proj(b, st, proj_rhs):
                lo = st * st_size
                out_j = op.tile([128, CT * st_size], F32, tag="out_j",
                                name=f"out_j_{b}_{st}", bufs=2)
                x_sl = op.tile([128, CT * st_size], BF16, tag="x_sl",
                               name=f"x_sl_{b}_{st}", bufs=2)
                nc.sync.dma_start(
                    x_sl[:],
                    x_in[b, :, :, lo:lo + st_size].transpose([1, 0, 2]))
                for ot in range(OT):
                    y_p = mmp.tile([128, st_size], F32, tag="mm_p",
                                   name=f"y_p_{b}_{st}_{ot}")
                    for ct in range(CT):
                        nc.tensor.matmul(
                            y_p[:],
                            pw_t[:, ct * C + ot * 128:ct * C + (ot + 1) * 128],
                            proj_rhs[:, ct * st_size:(ct + 1) * st_size],
                            start=(ct == 0), stop=(ct == CT - 1))
                    nc.vector.scalar_tensor_tensor(
                        out_j[:, ot * st_size:(ot + 1) * st_size], y_p[:],
                        pb_t[:, ot:ot + 1],
                        x_sl[:, ot * st_size:(ot + 1) * st_size],
                        mybir.AluOpType.add, mybir.AluOpType.add)
                nc.sync.dma_start(out_ext[b, st], out_j[:])

            # software-pipelined emission: proj of iter i lands between the
            # head groups of iter i+1, hiding the reciprocal roundtrip
            iters = [(b, st) for b in range(B) for st in range(nst)]
            prev = None
            for (b, st) in iters:
                q_s = emit_q(b, st)
                proj_rhs = mp.tile([128, CT * st_size], BF16, tag="proj_rhs",
                                   bufs=3, name=f"proj_rhs_{b}_{st}")
                rs8_d = rsd.tile([8, st_size], F32, tag="rs8_d",
                                 name=f"rs8_d_{b}_{st}")
                o_list = []
                emit_head_group(b, st, 0, q_s, rs8_d, o_list)
                if prev is not None:
                    emit_proj(prev[0], prev[1], prev[2])
                emit_head_group(b, st, 1, q_s, rs8_d, o_list)
                emit_normalize(b, st, rs8_d, o_list, proj_rhs)
                prev = (b, st, proj_rhs)
            emit_proj(prev[0], prev[1], prev[2])
    return nc


def prep_inputs(x, context, gn_w, gn_b, ln_w, ln_b, q_w, q_b, k_w, k_b,
                v_w, v_b, proj_w, proj_b, s_loc):
    """Host-side shard + layout prep. Returns in_maps for the 8 cores."""
    scale = HD ** -0.5
    qwT = (q_w.astype(np.float64) * scale).T.astype(np.float32)
    kwT = (k_w.astype(np.float64) * ln_w.astype(np.float64)[None, :]).T.astype(np.float32)
    vwT = (v_w.astype(np.float64) * ln_w.astype(np.float64)[None, :]).T.astype(np.float32)
    pwT = proj_w.T.astype(np.float32)
    kb_eff = (k_b + ln_b @ k_w.T).astype(np.float32)
    vb_eff = (v_b + ln_b @ v_w.T).astype(np.float32)
    pb_eff = (proj_b + vb_eff @ proj_w.T).astype(np.float32)
    qb_eff = (q_b * scale).astype(np.float32)

    gnw8 = np.empty((128, B * CT), np.float32)
    gnb8 = np.empty((128, B * CT), np.float32)
    for b in range(B):
        for t in range(CT):
            gnw8[:, b * CT + t] = gn_w[t * 128:(t + 1) * 128]
            gnb8[:, b * CT + t] = gn_b[t * 128:(t + 1) * 128]

    sel8 = np.zeros((8, 8 * 64), np.float32)
    for k in range(8):
        sel8[k, k * 64:(k + 1) * 64] = 1.0
    rmask = (np.arange(64)[:, None] % 8 == np.arange(8)[None, :]).astype(np.float32)

    def cols(wt, nt):     # [C_in, C_out] -> [128, nt*C_out]
        return np.ascontiguousarray(
            wt.reshape(nt, 128, C).transpose(1, 0, 2).reshape(128, nt * C))

    wpack = np.concatenate(
        [cols(qwT, CT), cols(kwT, KCT), cols(vwT, KCT), cols(pwT, CT)],
        axis=1).astype(BF)
    bpack = np.concatenate(
        [qb_eff.reshape(CT, 128).T, kb_eff.reshape(CT, 128).T,
         pb_eff.reshape(CT, 128).T, gnw8, gnb8], axis=1).astype(np.float32)
    ctxp = np.ascontiguousarray(
        context.reshape(B, LT, 128, CTX).transpose(2, 0, 1, 3)
        .reshape(128, B * LT * CTX)).astype(np.float32)

    shared = {
        "wpack": wpack,
        "bpack": np.ascontiguousarray(bpack),
        "ctx": ctxp,
        "sel8": sel8.astype(BF),
        "rmask": rmask,
    }
    xr = x.reshape(B, C, S)
    in_maps = []
    for i in range(NCORES):
        xs = np.ascontiguousarray(xr[:, :, i * s_loc:(i + 1) * s_loc])
        m = dict(shared)
        m["x"] = xs.reshape(B, CT, 128, s_loc).astype(BF)
        in_maps.append(m)
    return in_maps


def _install_prof_shim():
    """Register the NTFF profile hook that this container's antenv lacks."""
    import types
    import antenv

    if "antenv.axon_hooks" not in sys.modules:
        mod = types.ModuleType("antenv.axon_hooks")
        mod._hook = None
        mod.set_axon_ntff_profile_hook = lambda h: setattr(mod, "_hook", h)
        mod.get_axon_ntff_profile_hook = lambda: mod._hook
        sys.modules["antenv.axon_hooks"] = mod
        antenv.axon_hooks = mod
    sys.path.insert(0, "/root/.axon_site")
    from trn_agent_boot.trn_boot import _ntff_profile_via_ctypes
    from antenv.axon_hooks import set_axon_ntff_profile_hook

    hook = _ntff_profile_via_ctypes("/opt/axon/libaxon_pjrt.so")
    assert hook is not None
    set_axon_ntff_profile_hook(hook)
    from concourse import bass_utils as bu
    bu.upload_artifacts = lambda tmpdir: tmpdir


def kernel(x, context, gn_w, gn_b, ln_w, ln_b, q_w, q_b, k_w, k_b,
           v_w, v_b, proj_w, proj_b):
    import os
    s_loc = S // NCORES
    st_size = 512
    in_maps = prep_inputs(x, context, gn_w, gn_b, ln_w, ln_b, q_w, q_b,
                          k_w, k_b, v_w, v_b, proj_w, proj_b, s_loc)
    nc = bacc.Bacc("TRN2", target_bir_lowering=False, debug=False,
                   num_devices=NCORES)
    build(nc, s_loc, st_size)
    nc.compile()
    trace = bool(os.environ.get("KPROF"))
    if trace:
        try:
            _install_prof_shim()
        except Exception as e:
            print(f"profiling shim unavailable ({e}); running untraced")
            trace = False
    try:
        res = run_bass_kernel_spmd(nc, in_maps, list(range(NCORES)),
                                   trace=trace,
                                   tmpdir=os.environ.get("KPROF_DIR"))
    except Exception:
        if not trace:
            raise
        print("traced run failed; retrying untraced")
        res = run_bass_kernel_spmd(nc, in_maps, list(range(NCORES)))
    if trace and res.exec_time_ns is not None:
        print(f"HW exec time: {res.exec_time_ns} ns")
    nst = s_loc // st_size
    out = np.empty((B, C, S), np.float32)
    for i in range(NCORES):
        r = res.results[i]["out"].reshape(B, nst, 128, CT, st_size)
        r = r.transpose(0, 3, 2, 1, 4).reshape(B, C, s_loc)
        out[:, :, i * s_loc:(i + 1) * s_loc] = r
    return out.reshape(B, C, D, H, W)


# revision 43
# speedup vs baseline: 1.0295x; 1.0295x over previous
"""CrossAttentionBlock3D on 8 TRN2 NeuronCores — sequence-parallel Bass kernel.

Sharding: the 32768 spatial tokens are split 8x4096 across cores. GroupNorm
statistics are the only cross-core dependency (one 64-float AllReduce).
Everything else (LN, K/V projections over the tiny context, Q/attention/proj
for the local tokens) is computed locally; context-side work is replicated.

Compute dtype: bf16 matmuls with f32 PSUM accumulation (validated end-to-end
rel err ~3e-3 vs the f32 reference).
"""
import sys

sys.path.insert(0, "/opt/trn_rl_repo")

import numpy as np
import ml_dtypes

from concourse import bass, bacc, tile, mybir, masks
from concourse.bass_utils import run_bass_kernel_spmd

F32 = mybir.dt.float32
BF16 = mybir.dt.bfloat16
BF = ml_dtypes.bfloat16

B, C, D, H, W = 2, 512, 32, 32, 32
S = D * H * W              # 32768
L, CTX = 256, 768
NH, HD, G = 8, 64, 8
EPS = 1e-5
NCORES = 8
CT, OT = 4, 4              # channel tiles (C = 4*128)
KCT = 6                    # ctx channel tiles (CTX = 6*128)
LT = 2                     # L = 2*128


def build(nc, s_loc, st_size):
    """Emit the per-core Tile program. s_loc: local tokens; st_size: S-tile."""
    nst = s_loc // st_size
    n_gn = (C // G) * S    # global elems per (b, g) group

    WCOLS = 2 * CT * C + 2 * KCT * C          # qw | kw | vw | pw packed
    BCOLS = 3 * CT + 2 * B * CT               # qb | kb | pb | gnw8 | gnb8
    x_in = nc.dram_tensor("x", [B, CT, 128, s_loc], BF16, kind="ExternalInput")
    ctx_in = nc.dram_tensor("ctx", [128, B * LT * CTX], F32, kind="ExternalInput")
    w_in = nc.dram_tensor("wpack", [128, WCOLS], BF16, kind="ExternalInput")
    b_in = nc.dram_tensor("bpack", [128, BCOLS], F32, kind="ExternalInput")
    sel_in = nc.dram_tensor("sel8", [8, 8 * 64], BF16, kind="ExternalInput")
    rm_in = nc.dram_tensor("rmask", [64, 8], F32, kind="ExternalInput")
    out_ext = nc.dram_tensor("out", [B, nst, 128, CT * st_size], F32,
                             kind="ExternalOutput")

    from contextlib import ExitStack
    with tile.TileContext(nc) as tc, ExitStack() as es:
        wp = es.enter_context(tc.tile_pool(name="wp", bufs=1))
        dram = es.enter_context(tc.tile_pool(name="dram", bufs=1, space="DRAM"))

        # ---- persistent SBUF tensors ----
        w_all = wp.tile([128, WCOLS], BF16, tag="wpack")
        b_all = wp.tile([128, BCOLS], F32, tag="bpack")
        qw_t = w_all[:, 0:CT * C]
        kw_t = w_all[:, CT * C:CT * C + KCT * C]
        vw_t = w_all[:, CT * C + KCT * C:CT * C + 2 * KCT * C]
        pw_t = w_all[:, CT * C + 2 * KCT * C:WCOLS]
        qb_t = b_all[:, 0:CT]
        kb_t = b_all[:, CT:2 * CT]
        pb_t = b_all[:, 2 * CT:3 * CT]
        gnw_t = b_all[:, 3 * CT:3 * CT + B * CT]
        gnb_t = b_all[:, 3 * CT + B * CT:BCOLS]
        ctxT_all = wp.tile([128, B * KCT * L], BF16, tag="ctxT")
        kT_all = wp.tile([128, B * CT * L], BF16, tag="kT")
        v_all = wp.tile([128, B * LT * (NH * (HD + 1))], BF16, tag="v")
        ones_t = wp.tile([1, 64], F32, tag="ones")
        ident = wp.tile([128, 128], BF16, tag="ident")
        stats_s = wp.tile([128, 16], F32, tag="stats")
        h_all = wp.tile([128, B * CT * s_loc], BF16, tag="h_all")
        a_pc = wp.tile([128, B * CT], F32, tag="a_pc")
        bias_pc = wp.tile([128, B * CT], F32, tag="bias_pc")

        sel8_t = wp.tile([8, 8 * 64], BF16, tag="sel8")
        rmask_t = wp.tile([64, 8], F32, tag="rmask")
        nc.vector.memset(ones_t[:], 1.0)
        masks.make_identity(nc, ident[:])

        # ---- GroupNorm partial stats (local) ----
        with tc.tile_pool(name="setup", bufs=1) as sp, \
             tc.tile_pool(name="setup_ps", bufs=2, space="PSUM") as spp:
            warm_in = dram.tile([8, 4], F32, tag="warm_in")
            warm_out = dram.tile([64, 4], F32, tag="warm_out")
            warm_s = sp.tile([8, 4], F32, tag="warm_s")
            nc.vector.memset(warm_s[:], 0.0)
            nc.gpsimd.dma_start(warm_in[:], warm_s[:])
            nc.gpsimd.collective_compute(
                "AllGather", mybir.AluOpType.bypass,
                replica_groups=[list(range(NCORES))],
                ins=[warm_in.opt()], outs=[warm_out.opt()])
            for b in range(B):
                for t in range(CT):
                    col = b * CT + t
                    x_t = sp.tile([128, s_loc], BF16, tag="x_t", bufs=3,
                                  name=f"x_t_{col}")
                    nc.sync.dma_start(x_t[:], x_in[b, t])
                    nc.vector.tensor_reduce(
                        stats_s[:, col:col + 1], x_t[:], mybir.AxisListType.X,
                        mybir.AluOpType.add)
                    nc.scalar.activation(
                        h_all[:, col * s_loc:(col + 1) * s_loc], x_t[:],
                        mybir.ActivationFunctionType.Square,
                        accum_out=stats_s[:, 8 + col:9 + col])
            nc.sync.dma_start(w_all[:], w_in[:])
            nc.sync.dma_start(b_all[:], b_in[:])
            nc.sync.dma_start(sel8_t[:], sel_in[:])
            nc.sync.dma_start(rmask_t[:], rm_in[:])
            ctxf = sp.tile([128, B * LT * CTX], F32, tag="ctxf")
            nc.sync.dma_start(ctxf[:], ctx_in[:])

            mask2 = sp.tile([128, 2], F32, tag="mask2")
            nc.vector.memset(mask2[:, :], 0.0)
            nc.vector.memset(mask2[0:64, 0:1], 1.0)
            nc.vector.memset(mask2[64:128, 1:2], 1.0)
            st_p = spp.tile([8, 4], F32, tag="st_p", bufs=1)
            nc.tensor.matmul(st_p[:, 0:2], stats_s[:, 0:8], mask2[:],
                             start=True, stop=True)
            nc.tensor.matmul(st_p[:, 2:4], stats_s[:, 8:16], mask2[:],
                             start=True, stop=True)
            red_s = sp.tile([8, 4], F32, tag="red_s")
            nc.vector.tensor_copy(red_s[:], st_p[:])

            cc_in = dram.tile([8, 4], F32, tag="cc_in")
            cc_ag = dram.tile([64, 4], F32, tag="cc_ag")
            nc.gpsimd.dma_start(cc_in[:], red_s[:])
            nc.gpsimd.collective_compute(
                "AllGather", mybir.AluOpType.bypass,
                replica_groups=[list(range(NCORES))],
                ins=[cc_in.opt()], outs=[cc_ag.opt()])
            ag_s = sp.tile([64, 4], F32, tag="ag_s")
            nc.gpsimd.dma_start(ag_s[:], cc_ag[:])
            sum_p = spp.tile([2, 8], F32, tag="sum_p", bufs=1)
            sq_p = spp.tile([2, 8], F32, tag="sq_p", bufs=1)
            nc.tensor.matmul(sum_p[:], ag_s[:, 0:2], rmask_t[:],
                             start=True, stop=True)
            nc.tensor.matmul(sq_p[:], ag_s[:, 2:4], rmask_t[:],
                             start=True, stop=True)

            # per-(b,g) mean / rstd, laid out [2 halves, 8 (b,t)]
            mu8 = sp.tile([2, 8], F32, tag="mu8")
            rstd8 = sp.tile([2, 8], F32, tag="rstd8")
            ex28 = sp.tile([2, 8], F32, tag="ex28")
            var8 = sp.tile([2, 8], F32, tag="var8")
            sd8 = sp.tile([2, 8], F32, tag="sd8")
            eps8 = sp.tile([2, 1], F32, tag="eps8")
            nc.vector.memset(eps8[:], EPS)
            nc.vector.tensor_scalar_mul(mu8[:], sum_p[:], 1.0 / n_gn)
            nc.vector.tensor_scalar_mul(ex28[:], sq_p[:], 1.0 / n_gn)
            nc.vector.scalar_tensor_tensor(
                var8[:], mu8[:], -1.0, mu8[:],
                mybir.AluOpType.mult, mybir.AluOpType.mult)
            nc.vector.tensor_add(var8[:], var8[:], ex28[:])
            nc.scalar.activation(sd8[:], var8[:],
                                 mybir.ActivationFunctionType.Sqrt, bias=eps8[:])
            nc.vector.reciprocal(rstd8[:], sd8[:])

            # broadcast [2,8] -> [128, 8] via DRAM bounce + stride-0 DMA
            mu_d = dram.tile([2, 8], F32, tag="mu_d")
            rstd_d = dram.tile([2, 8], F32, tag="rstd_d")
            nc.gpsimd.dma_start(mu_d[:], mu8[:])
            nc.gpsimd.dma_start(rstd_d[:], rstd8[:])
            mu_bc = sp.tile([128, 8], F32, tag="mu_bc")
            rstd_bc = sp.tile([128, 8], F32, tag="rstd_bc")
            nc.gpsimd.dma_start(
                mu_bc[:], mu_d[:, :].unsqueeze(1).broadcast_to((2, 64, 8)))
            nc.gpsimd.dma_start(
                rstd_bc[:], rstd_d[:, :].unsqueeze(1).broadcast_to((2, 64, 8)))

            # per-channel affine: h = a*x + bias
            nc.vector.tensor_mul(a_pc[:], rstd_bc[:], gnw_t[:])
            tmp_bc = sp.tile([128, 8], F32, tag="tmp_bc")
            nc.vector.tensor_mul(tmp_bc[:], mu_bc[:], a_pc[:])
            nc.vector.tensor_sub(bias_pc[:], gnb_t[:], tmp_bc[:])

            # ---- h = a*x + bias for the whole shard (bf16) ----
            # b0 first-tile slices first so q(0,0) unblocks immediately
            for b in range(B):
                for ct in range(CT):
                    col = b * CT + ct
                    x_t2 = sp.tile([128, s_loc], BF16, tag="x_t", bufs=3,
                                   name=f"x_t2_{col}")
                    nc.sync.dma_start(x_t2[:], x_in[b, ct])
                    if b == 0:
                        nc.vector.tensor_scalar(
                            h_all[:, col * s_loc:col * s_loc + st_size],
                            x_t2[:, 0:st_size],
                            a_pc[:, col:col + 1], bias_pc[:, col:col + 1],
                            mybir.AluOpType.mult, mybir.AluOpType.add)
                        nc.vector.tensor_scalar(
                            h_all[:, col * s_loc + st_size:(col + 1) * s_loc],
                            x_t2[:, st_size:],
                            a_pc[:, col:col + 1], bias_pc[:, col:col + 1],
                            mybir.AluOpType.mult, mybir.AluOpType.add)
                    else:
                        nc.vector.tensor_scalar(
                            h_all[:, col * s_loc:(col + 1) * s_loc],
                            x_t2[:],
                            a_pc[:, col:col + 1], bias_pc[:, col:col + 1],
                            mybir.AluOpType.mult, mybir.AluOpType.add)

            # ---- context path: LN + transpose + K/V ----
            ctxn = sp.tile([128, B * LT * CTX], BF16, tag="ctxn")
            eps128 = sp.tile([128, 1], F32, tag="eps128")
            nc.vector.memset(eps128[:], EPS)
            for b in range(B):
                for lt in range(LT):
                    cs = ctxf[:, (b * LT + lt) * CTX:(b * LT + lt + 1) * CTX]
                    cs1 = sp.tile([128, 1], F32, tag="cs1", bufs=2)
                    cs2 = sp.tile([128, 1], F32, tag="cs2", bufs=2)
                    csq = sp.tile([128, CTX], F32, tag="csq", bufs=2)
                    nc.vector.tensor_reduce(cs1[:], cs, mybir.AxisListType.X,
                                            mybir.AluOpType.add)
                    nc.scalar.activation(csq[:], cs,
                                         mybir.ActivationFunctionType.Square,
                                         accum_out=cs2[:])
                    cmu = sp.tile([128, 1], F32, tag="cmu", bufs=2)
                    cex2 = sp.tile([128, 1], F32, tag="cex2", bufs=2)
                    cvar = sp.tile([128, 1], F32, tag="cvar", bufs=2)
                    csd = sp.tile([128, 1], F32, tag="csd", bufs=2)
                    crstd = sp.tile([128, 1], F32, tag="crstd", bufs=2)
                    cnm = sp.tile([128, 1], F32, tag="cnm", bufs=2)
                    nc.vector.tensor_scalar_mul(cmu[:], cs1[:], 1.0 / CTX)
                    nc.vector.tensor_scalar_mul(cex2[:], cs2[:], 1.0 / CTX)
                    nc.vector.scalar_tensor_tensor(
                        cvar[:], cmu[:], -1.0, cmu[:],
                        mybir.AluOpType.mult, mybir.AluOpType.mult)
                    nc.vector.tensor_add(cvar[:], cvar[:], cex2[:])
                    nc.scalar.activation(csd[:], cvar[:],
                                         mybir.ActivationFunctionType.Sqrt,
                                         bias=eps128[:])
                    nc.vector.reciprocal(crstd[:], csd[:])
                    nc.vector.scalar_tensor_tensor(
                        cnm[:], cmu[:], -1.0, crstd[:],
                        mybir.AluOpType.mult, mybir.AluOpType.mult)
                    nc.vector.tensor_scalar(
                        ctxn[:, (b * LT + lt) * CTX:(b * LT + lt + 1) * CTX],
                        cs, crstd[:], cnm[:],
                        mybir.AluOpType.mult, mybir.AluOpType.add)

            # transpose ctxn -> ctxT_all  [128ctx, L] per (b, kct)
            for b in range(B):
                for lt in range(LT):
                    for ct in range(KCT):
                        tp_p = spp.tile([128, 128], BF16, tag="tp_p")
                        nc.tensor.transpose(
                            tp_p[:],
                            ctxn[:, (b * LT + lt) * CTX + ct * 128:
                                 (b * LT + lt) * CTX + (ct + 1) * 128],
                            ident[:])
                        nc.scalar.copy(
                            ctxT_all[:, (b * KCT + ct) * L + lt * 128:
                                     (b * KCT + ct) * L + (lt + 1) * 128],
                            tp_p[:])

            # kT[b, ot] [128, L]
            for b in range(B):
                for ot in range(OT):
                    k_p = spp.tile([128, L], F32, tag="k_p", bufs=1)
                    for ct in range(KCT):
                        nc.tensor.matmul(
                            k_p[:],
                            kw_t[:, ct * C + ot * 128:ct * C + (ot + 1) * 128],
                            ctxT_all[:, (b * KCT + ct) * L:(b * KCT + ct + 1) * L],
                            start=(ct == 0), stop=(ct == KCT - 1))
                    nc.scalar.activation(
                        kT_all[:, (b * CT + ot) * L:(b * CT + ot + 1) * L],
                        k_p[:], mybir.ActivationFunctionType.Identity,
                        bias=kb_t[:, ot:ot + 1])

            # v'[b, lt] [128, NH*(HD+1)]  (per-head ones column appended)
            VW = NH * (HD + 1)
            for b in range(B):
                for lt in range(LT):
                    v_p = spp.tile([128, C], F32, tag="v_p", bufs=1)
                    for ct in range(KCT):
                        nc.tensor.matmul(
                            v_p[:],
                            ctxT_all[:, (b * KCT + ct) * L + lt * 128:
                                     (b * KCT + ct) * L + (lt + 1) * 128],
                            vw_t[:, ct * C:(ct + 1) * C],
                            start=(ct == 0), stop=(ct == KCT - 1))
                    vs = v_all[:, (b * LT + lt) * VW:(b * LT + lt + 1) * VW]
                    nc.scalar.copy(
                        vs.rearrange("p (h e) -> p h e", e=HD + 1)[:, :, 0:HD],
                        v_p[:])
                    nc.vector.memset(
                        vs.rearrange("p (h e) -> p h e", e=HD + 1)[:, :, HD:HD + 1],
                        1.0)

        # ---- main attention loop (software-pipelined) ----
        with tc.tile_pool(name="mp", bufs=2) as mp, \
             tc.tile_pool(name="op", bufs=3) as op, \
             tc.tile_pool(name="mm_ps", bufs=2, space="PSUM") as mmp, \
             tc.tile_pool(name="z_ps", bufs=2, space="PSUM") as zp, \
             tc.tile_pool(name="o_ps", bufs=1, space="PSUM") as opp, \
             tc.tile_pool(name="rsb_ps", bufs=1, space="PSUM") as rbp, \
             tc.tile_pool(name="rs_dram", bufs=3, space="DRAM") as rsd:

            def emit_q(b, st):
                lo = st * st_size
                q_s = mp.tile([128, CT * st_size], BF16, tag="q_s",
                              name=f"q_s_{b}_{st}")
                for ot in range(OT):
                    q_p = mmp.tile([128, st_size], F32, tag="mm_p",
                                   name=f"q_p_{b}_{st}_{ot}")
                    for ct in range(CT):
                        nc.tensor.matmul(
                            q_p[:],
                            qw_t[:, ct * C + ot * 128:ct * C + (ot + 1) * 128],
                            h_all[:, (b * CT + ct) * s_loc + lo:
                                  (b * CT + ct) * s_loc + lo + st_size],
                            start=(ct == 0), stop=(ct == CT - 1))
                    nc.scalar.activation(
                        q_s[:, ot * st_size:(ot + 1) * st_size], q_p[:],
                        mybir.ActivationFunctionType.Identity,
                        bias=qb_t[:, ot:ot + 1])
                return q_s

            def emit_head_group(b, st, hg, q_s, rs8_d, o_list):
                for hj in range(4):
                    hh = hg * 4 + hj
                    ko, po = hh // 2, (hh % 2) * 64
                    p_t = mp.tile([128, 2 * st_size], BF16, tag="p_t", bufs=3,
                                  name=f"p_t_{b}_{st}_{hh}")
                    z_p = zp.tile([128, 2 * st_size], F32, tag="z_p",
                                  name=f"z_p_{b}_{st}_{hh}")
                    for lh in range(LT):
                        nc.tensor.matmul(
                            z_p[:, lh * st_size:(lh + 1) * st_size],
                            kT_all[po:po + 64,
                                   (b * CT + ko) * L + lh * 128:
                                   (b * CT + ko) * L + (lh + 1) * 128],
                            q_s[po:po + 64, ko * st_size:(ko + 1) * st_size],
                            start=True, stop=True)
                    nc.scalar.activation(p_t[:], z_p[:],
                                         mybir.ActivationFunctionType.Exp)
                    o_p = opp.tile([HD + 1, st_size], F32, tag="o_p",
                                   name=f"o_p_{b}_{st}_{hh}")
                    for lh in range(LT):
                        nc.tensor.matmul(
                            o_p[:],
                            v_all[:, (b * LT + lh) * VW + hh * (HD + 1):
                                  (b * LT + lh) * VW + (hh + 1) * (HD + 1)],
                            p_t[:, lh * st_size:(lh + 1) * st_size],
                            start=(lh == 0), stop=(lh == LT - 1))
                    o_s = mp.tile([HD + 1, st_size], F32, tag="o_s", bufs=10,
                                  name=f"o_s_{b}_{st}_{hh}")
                    if hj % 2 == 0:
                        nc.scalar.copy(o_s[:], o_p[:])
                    else:
                        nc.vector.tensor_copy(o_s[:], o_p[:])
                    o_list.append(o_s)
                    nc.gpsimd.dma_start(rs8_d[hh:hh + 1, :], o_s[HD:HD + 1, :])

            def emit_normalize(b, st, rs8_d, o_list, proj_rhs):
                rs8_s = mp.tile([8, st_size], F32, tag="rs8_s",
                                name=f"rs8_s_{b}_{st}")
                nc.gpsimd.dma_start(rs8_s[:], rs8_d[:])
                rec8 = mp.tile([8, st_size], F32, tag="rec8",
                               name=f"rec8_{b}_{st}")
                nc.vector.reciprocal(rec8[:], rs8_s[:])
                rec8b = mp.tile([8, st_size], BF16, tag="rec8b",
                                name=f"rec8b_{b}_{st}")
                nc.vector.tensor_copy(rec8b[:], rec8[:])
                for hh in range(NH):
                    ko, po = hh // 2, (hh % 2) * 64
                    rsb_p = rbp.tile([64, st_size], F32, tag="rsb_p",
                                     name=f"rsb_p_{b}_{st}_{hh}")
                    nc.tensor.matmul(
                        rsb_p[:], sel8_t[:, hh * 64:(hh + 1) * 64],
                        rec8b[:], start=True, stop=True)
                    nc.vector.tensor_tensor(
                        proj_rhs[po:po + 64, ko * st_size:(ko + 1) * st_size],
                        o_list[hh][0:HD, :], rsb_p[:], mybir.AluOpType.mult)

            def emit_proj(b, st, proj_rhs):
                lo = st * st_size
                out_j = op.tile([128, CT * st_size], F32, tag="out_j",
                                name=f"out_j_{b}_{st}", bufs=2)
                x_sl = op.tile([128, CT * st_size], BF16, tag="x_sl",
                               name=f"x_sl_{b}_{st}", bufs=2)
                nc.sync.dma_start(
                    x_sl[:],
                    x_in[b, :, :, lo:lo + st_size].transpose([1, 0, 2]))
                for ot in range(OT):
                    y_p = mmp.tile([128, st_size], F32, tag="mm_p",
                                   name=f"y_p_{b}_{st}_{ot}")
                    for ct in range(CT):
                        nc.tensor.matmul(
                            y_p[:],
                            pw_t[:, ct * C + ot * 128:ct * C + (ot + 1) * 128],
                            proj_rhs[:, ct * st_size:(ct + 1) * st_size],
                            start=(ct == 0), stop=(ct == CT - 1))
                    nc.vector.scalar_tensor_tensor(
                        out_j[:, ot * st_size:(ot + 1) * st_size], y_p[:],
                        pb_t[:, ot:ot + 1],
                        x_sl[:, ot * st_size:(ot + 1) * st_size],
                        mybir.AluOpType.add, mybir.AluOpType.add)
                nc.sync.dma_start(out_ext[b, st], out_j[:])

            # software-pipelined emission: proj of iter i lands between the
            # head groups of iter i+1, hiding the reciprocal roundtrip
            iters = [(b, st) for b in range(B) for st in range(nst)]
            prev = None
            for (b, st) in iters:
                q_s = emit_q(b, st)
                proj_rhs = mp.tile([128, CT * st_size], BF16, tag="proj_rhs",
                                   name=f"proj_rhs_{b}_{st}")
                rs8_d = rsd.tile([8, st_size], F32, tag="rs8_d",
                                 name=f"rs8_d_{b}_{st}")
                o_list = []
                emit_head_group(b, st, 0, q_s, rs8_d, o_list)
                if prev is not None:
                    emit_proj(prev[0], prev[1], prev[2])
                emit_head_group(b, st, 1, q_s, rs8_d, o_list)
                emit_normalize(b, st, rs8_d, o_list, proj_rhs)
                prev = (b, st, proj_rhs)
            emit_proj(prev[0], prev[1], prev[2])
    return nc


def prep_inputs(x, context, gn_w, gn_b, ln_w, ln_b, q_w, q_b, k_w, k_b,
                v_w, v_b, proj_w, proj_b, s_loc):
    """Host-side shard + layout prep. Returns in_maps for the 8 cores."""
    scale = HD ** -0.5
    qwT = (q_w.astype(np.float64) * scale).T.astype(np.float32)
    kwT = (k_w.astype(np.float64) * ln_w.astype(np.float64)[None, :]).T.astype(np.float32)
    vwT = (v_w.astype(np.float64) * ln_w.astype(np.float64)[None, :]).T.astype(np.float32)
    pwT = proj_w.T.astype(np.float32)
    kb_eff = (k_b + ln_b @ k_w.T).astype(np.float32)
    vb_eff = (v_b + ln_b @ v_w.T).astype(np.float32)
    pb_eff = (proj_b + vb_eff @ proj_w.T).astype(np.float32)
    qb_eff = (q_b * scale).astype(np.float32)

    gnw8 = np.empty((128, B * CT), np.float32)
    gnb8 = np.empty((128, B * CT), np.float32)
    for b in range(B):
        for t in range(CT):
            gnw8[:, b * CT + t] = gn_w[t * 128:(t + 1) * 128]
            gnb8[:, b * CT + t] = gn_b[t * 128:(t + 1) * 128]

    sel8 = np.zeros((8, 8 * 64), np.float32)
    for k in range(8):
        sel8[k, k * 64:(k + 1) * 64] = 1.0
    rmask = (np.arange(64)[:, None] % 8 == np.arange(8)[None, :]).astype(np.float32)

    def cols(wt, nt):     # [C_in, C_out] -> [128, nt*C_out]
        return np.ascontiguousarray(
            wt.reshape(nt, 128, C).transpose(1, 0, 2).reshape(128, nt * C))

    wpack = np.concatenate(
        [cols(qwT, CT), cols(kwT, KCT), cols(vwT, KCT), cols(pwT, CT)],
        axis=1).astype(BF)
    bpack = np.concatenate(
        [qb_eff.reshape(CT, 128).T, kb_eff.reshape(CT, 128).T,
         pb_eff.reshape(CT, 128).T, gnw8, gnb8], axis=1).astype(np.float32)
    ctxp = np.ascontiguousarray(
        context.reshape(B, LT, 128, CTX).transpose(2, 0, 1, 3)
        .reshape(128, B * LT * CTX)).astype(np.float32)

    shared = {
        "wpack": wpack,
        "bpack": np.ascontiguousarray(bpack),
        "ctx": ctxp,
        "sel8": sel8.astype(BF),
        "rmask": rmask,
    }
    xr = x.reshape(B, C, S)
    in_maps = []
    for i in range(NCORES):
        xs = np.ascontiguousarray(xr[:, :, i * s_loc:(i + 1) * s_loc])
        m = dict(shared)
        m["x"] = xs.reshape(B, CT, 128, s_loc).astype(BF)
        in_maps.append(m)
    return in_maps


def _install_prof_shim():
    """Register the NTFF profile hook that this container's antenv lacks."""
    import types
    import antenv

    if "antenv.axon_hooks" not in sys.modules:
        mod = types.ModuleType("antenv.axon_hooks")
        mod._hook = None
        mod.set_axon_ntff_profile_hook = lambda h: setattr(mod, "_hook", h)
        mod.get_axon_ntff_profile_hook = lambda: mod._hook
        sys.modules["antenv.axon_hooks"] = mod
        antenv.axon_hooks = mod
    sys.path.insert(0, "/root/.axon_site")
    from trn_agent_boot.trn_boot import _ntff_profile_via_ctypes
    from antenv.axon_hooks import set_axon_ntff_profile_hook

    hook = _ntff_profile_via_ctypes("/opt/axon/libaxon_pjrt.so")
    assert hook is not None
    set_axon_ntff_profile_hook(hook)
    from concourse import bass_utils as bu
    bu.upload_artifacts = lambda tmpdir: tmpdir


def kernel(x, context, gn_w, gn_b, ln_w, ln_b, q_w, q_b, k_w, k_b,
           v_w, v_b, proj_w, proj_b):
    import os
    s_loc = S // NCORES
    st_size = 512
    in_maps = prep_inputs(x, context, gn_w, gn_b, ln_w, ln_b, q_w, q_b,
                          k_w, k_b, v_w, v_b, proj_w, proj_b, s_loc)
    nc = bacc.Bacc("TRN2", target_bir_lowering=False, debug=False,
                   num_devices=NCORES)
    build(nc, s_loc, st_size)
    nc.compile()
    trace = bool(os.environ.get("KPROF"))
    if trace:
        try:
            _install_prof_shim()
        except Exception as e:
            print(f"profiling shim unavailable ({e}); running untraced")
            trace = False
    try:
        res = run_bass_kernel_spmd(nc, in_maps, list(range(NCORES)),
                                   trace=trace,
                                   tmpdir=os.environ.get("KPROF_DIR"))
    except Exception:
        if not trace:
            raise
        print("traced run failed; retrying untraced")
        res = run_bass_kernel_spmd(nc, in_maps, list(range(NCORES)))
    if trace and res.exec_time_ns is not None:
        print(f"HW exec time: {res.exec_time_ns} ns")
    nst = s_loc // st_size
    out = np.empty((B, C, S), np.float32)
    for i in range(NCORES):
        r = res.results[i]["out"].reshape(B, nst, 128, CT, st_size)
        r = r.transpose(0, 3, 2, 1, 4).reshape(B, C, s_loc)
        out[:, :, i * s_loc:(i + 1) * s_loc] = r
    return out.reshape(B, C, D, H, W)
